# revision 4
# baseline (speedup 1.0000x reference)
"""DependencyBertSelfAttention Trainium2 kernel (v2).

Sharding: batch B=8 -> one batch element per NeuronCore (8 cores, SPMD).
Per core (full T=1024, C=768, H=12 heads, D=64):

ACT (exp/tanh) is the roofline engine (~200us of elementwise work that only
it can do), so the whole schedule is built to keep it saturated:

  - Projections are interleaved INTO the head loop (qk0 first, V blocks
    just-in-time before the PV matmuls that need them, remaining qk chunks
    one per head) so the first exp fires ~14us in instead of ~67us.
  - All matmul inputs are bf16 (host-converted): same PE throughput, half
    the DMA bytes and SBUF footprint.
  - dsa branch: DVE writes s*dep into [128,4096] bf16 quarter-tiles; ONE
    in-place exp per 4 s-blocks amortizes the ACT access latency.
  - dsa-PV is skewed by a half-head behind the osa flow so PE never blocks
    the ACT stream; per head ACT runs 8 pO exps + 2 merged pD exps + 1
    strided tanh back to back.
  - Gate work is spread through the loop: after head h's PV-normalize
    (DVE), tanh of just head h's od columns (strided AP) feeds per-head
    partial z reductions (DVE/Pool split) and incremental osa-dsa diffs
    (Pool), leaving only head 11's slice + 8 tiny gate chains in the tail.

No inter-core communication: each core's batch element is independent.
"""
import sys

sys.path.insert(0, "/opt/trn_rl_repo")

import numpy as np
import ml_dtypes
from contextlib import ExitStack, nullcontext

import concourse.bass as bass
import concourse.tile as tile
from concourse import bacc, mybir

B, T, C, H, D = 8, 1024, 768, 12, 64
CB = C // 128   # 6 channel partition-blocks
SB = T // 128   # 8 s/t blocks
NCORES = 8

F32 = mybir.dt.float32
BF16 = mybir.dt.bfloat16
AF = mybir.ActivationFunctionType
ALU = mybir.AluOpType


def build_nc(debug=False, repeat=1):
    nc = bacc.Bacc("TRN2", target_bir_lowering=False, debug=False,
                   num_devices=NCORES)

    xT_d = nc.dram_tensor("xT", [C, T], BF16, kind="ExternalInput").ap()
    wq_d = nc.dram_tensor("wq", [C, C], BF16, kind="ExternalInput").ap()
    wk_d = nc.dram_tensor("wk", [C, C], BF16, kind="ExternalInput").ap()
    wv_d = nc.dram_tensor("wv", [C, C], BF16, kind="ExternalInput").ap()
    bqs_d = nc.dram_tensor("bqs", [C, 1], F32, kind="ExternalInput").ap()
    bk_d = nc.dram_tensor("bk", [C, 1], F32, kind="ExternalInput").ap()
    bv_d = nc.dram_tensor("bv", [C], F32, kind="ExternalInput").ap()
    dep_d = nc.dram_tensor("dep", [T, T], BF16, kind="ExternalInput").ap()
    wg_d = nc.dram_tensor("wg", [2 * C], BF16, kind="ExternalInput").ap()
    nbg_d = nc.dram_tensor("nbg", [1], F32, kind="ExternalInput").ap()
    out_d = nc.dram_tensor("out", [T, C], F32, kind="ExternalOutput").ap()

    def bcast(src_ap, n_free):
        return bass.AP(tensor=src_ap.tensor, offset=src_ap.offset,
                       ap=[[0, 128], [1, n_free]])

    with tile.TileContext(nc, pool_alloc_mode="queue") as tc, ExitStack() as ctx:
      persist = ctx.enter_context(tc.tile_pool(name="persist", bufs=1))
      psS = ctx.enter_context(tc.tile_pool(name="psS", bufs=2, space="PSUM"))
      psPV = ctx.enter_context(tc.tile_pool(name="psPV", bufs=1, space="PSUM"))
      for _rep in range(repeat):
          # ---- persistent tiles
          qT = [persist.tile([128, T], BF16, tag=f"qT{i}", name=f"qT{i}") for i in range(CB)]
          kT = [persist.tile([128, T], BF16, tag=f"kT{i}", name=f"kT{i}") for i in range(CB)]
          vaug = [persist.tile([128, H * 65], BF16, tag=f"vaug{i}", name=f"vaug{i}") for i in range(SB)]
          dep_t = [persist.tile([128, T], BF16, tag=f"dep{i}", name=f"dep{i}") for i in range(SB)]
          wgb = persist.tile([128, 2 * C], BF16, tag="wgb", name="wgb")
          nbg_t = persist.tile([128, 1], F32, tag="nbg", name="nbg")
          bq_t = [persist.tile([128, 1], F32, tag=f"bq{i}", name=f"bq{i}") for i in range(CB)]
          bk_t = [persist.tile([128, 1], F32, tag=f"bk{i}", name=f"bk{i}") for i in range(CB)]
          # od: single big tile, col(tb, branch, h, d) = tb*1536 + branch*768 + h*64 + d
          od = persist.tile([128, SB * 2 * C], F32, tag="od", name="od")
          # per-tb diff (osa - dsa), built incrementally per head
          diff = [persist.tile([128, C], BF16, tag=f"diff{i}", name=f"diff{i}")
                  for i in range(SB)]
          # per-tb partial gate reductions, one column per head
          zparts = [persist.tile([128, H], F32, tag=f"zp{i}", name=f"zp{i}")
                    for i in range(SB)]

          # warm the ACT exp/tanh table set at t=0
          with tc.high_priority():
              warm = persist.tile([128, 1], F32, tag="warm", name="warm")
              nc.vector.memset(warm[:], 0.0)
              nc.scalar.activation(warm[:], warm[:], AF.Exp)

          # ================= DMAs =================
          sa = ctx.enter_context(tc.tile_pool(name="stageA", bufs=1))
          xT = [sa.tile([128, T], BF16, tag=f"xT{i}", name=f"xT{i}") for i in range(CB)]
          wts = {}
          for wname in ("q", "k", "v"):
              wts[wname] = [sa.tile([128, C], BF16, tag=f"w{wname}{i}", name=f"w{wname}{i}")
                            for i in range(CB)]
          bvb = sa.tile([128, C], F32, tag="bvb", name="bvb")

          for i in range(CB):
              nc.sync.dma_start(xT[i][:], xT_d[i * 128:(i + 1) * 128, :])
          for wname, w_d in (("q", wq_d), ("k", wk_d)):
              for i in range(CB):
                  nc.gpsimd.dma_start(wts[wname][i][:], w_d[i * 128:(i + 1) * 128, :])
          for i in range(CB):
              nc.sync.dma_start(bq_t[i][:], bqs_d[i * 128:(i + 1) * 128, :])
              nc.sync.dma_start(bk_t[i][:], bk_d[i * 128:(i + 1) * 128, :])
          for i in range(CB):
              nc.gpsimd.dma_start(wts["v"][i][:], wv_d[i * 128:(i + 1) * 128, :])
          nc.gpsimd.dma_start(bvb[:], bcast(bv_d, C))
          for i in range(SB):
              nc.sync.dma_start(dep_t[i][:], dep_d[i * 128:(i + 1) * 128, :])
          nc.gpsimd.dma_start(wgb[:], bcast(wg_d, 2 * C))
          nc.gpsimd.dma_start(nbg_t[:], bcast(nbg_d, 1))

          # ================= projection emitters =================
          def proj_qk(cb, on_pool=False):
              for dst, w, bias, scale in ((qT, wts["q"], bq_t, 0.125),
                                          (kT, wts["k"], bk_t, 1.0)):
                  ps = psS.tile([128, T], F32, tag="psS", name="psS")
                  for tch in range(2):
                      for kb in range(CB):
                          nc.tensor.matmul(
                              ps[:, tch * 512:(tch + 1) * 512],
                              w[kb][:, cb * 128:(cb + 1) * 128],
                              xT[kb][:, tch * 512:(tch + 1) * 512],
                              start=(kb == 0), stop=(kb == CB - 1))
                  eng = nc.gpsimd if on_pool else nc.vector
                  eng.tensor_scalar(dst[cb][:], ps[:], scale, bias[cb][:],
                                    ALU.mult, ALU.add)

          def proj_v(sb):
              va3 = vaug[sb][:].rearrange("p (h d) -> p h d", d=65)
              ps = psS.tile([128, T], F32, tag="psS", name="psSv")
              for ch, (n0, nw) in enumerate(((0, 512), (512, 256))):
                  for kb in range(CB):
                      nc.tensor.matmul(
                          ps[:, n0:n0 + nw],
                          xT[kb][:, sb * 128:(sb + 1) * 128],
                          wts["v"][kb][:, n0:n0 + nw],
                          start=(kb == 0), stop=(kb == CB - 1))
              ps3 = ps[:, 0:C].rearrange("p (h d) -> p h d", d=64)
              bv3 = bvb[:].rearrange("p (h d) -> p h d", d=64)
              nc.vector.tensor_add(va3[:, :, 0:64], ps3, bv3)
              nc.vector.memset(va3[:, :, 64:65], 1.0)

          # ================= head-loop pools =================
          pb = ctx.enter_context(tc.tile_pool(name="pP", bufs=6))
          sdp = ctx.enter_context(tc.tile_pool(name="sdp", bufs=2))
          rp = ctx.enter_context(tc.tile_pool(name="rp", bufs=8))
          todp = ctx.enter_context(tc.tile_pool(name="todp", bufs=2))

          # per-head state captured across the skewed pipeline
          ppvs = []   # 4 psum tiles [128,260], alive all heads
          state = {}  # h -> dict(sdt=[t0,t1], pOs=[...])

          def unit_front(h, sb):
              """scores + exp pO + dep-mult for (h, sb)."""
              hb, hoff = h // 2, (h % 2) * 64
              st = state[h]
              ps = psS.tile([128, T], F32, tag="psS", name="psS")
              for tch in range(2):
                  nc.tensor.matmul(
                      ps[:, tch * 512:(tch + 1) * 512],
                      kT[hb][hoff:hoff + 64, sb * 128:(sb + 1) * 128],
                      qT[hb][hoff:hoff + 64, tch * 512:(tch + 1) * 512],
                      start=True, stop=True)
              pO = pb.tile([128, T], BF16, tag="pO", name="pO")
              nc.scalar.activation(pO[:], ps[:], AF.Exp)
              st["pOs"].append(pO)
              j = sb % 4
              sdt = st["sdt"][sb // 4]
              nc.vector.tensor_mul(sdt[:, j * T:(j + 1) * T], ps[:], dep_t[sb][:])

          def pv(h, sb, p_tile, p_off, branch):
              """PV matmuls for one (head, s-block, branch) into ppv tiles."""
              base_b = 65 * branch
              for tbp in range(4):
                  ppv = ppvs[tbp]
                  for half in range(2):
                      tb = tbp * 2 + half
                      base = half * 130 + base_b
                      nc.tensor.matmul(
                          ppv[:, base:base + 65],
                          p_tile[:, p_off + tb * 128:p_off + (tb + 1) * 128],
                          vaug[sb][:, h * 65:h * 65 + 65],
                          start=(branch == 0 and sb == 0 and half == 0),
                          stop=(branch == 1 and sb == SB - 1))

          def sdexp(h, grp):
              sdt = state[h]["sdt"][grp]
              nc.scalar.activation(sdt[:], sdt[:], AF.Exp)

          def dsa_pv(h, grp):
              sdt = state[h]["sdt"][grp]
              for j in range(4):
                  sb = grp * 4 + j
                  pv(h, sb, sdt, j * T, branch=1)

          def norms(h):
              """reciprocal + normalized eviction into od, then strided tanh,
              partial z reductions, and incremental diff columns."""
              for tbp in range(4):
                  ppv = ppvs[tbp]
                  rec = rp.tile([128, 4], F32, tag="rec", name="rec")
                  den3 = ppv[:].rearrange("p (g d) -> p g d", d=65)[:, :, 64:65]
                  nc.vector.reciprocal(
                      rec[:].rearrange("p (g d) -> p g d", d=1), den3)
                  for half in range(2):
                      tb = tbp * 2 + half
                      base = half * 130
                      out3 = od[:, tb * 2 * C:(tb + 1) * 2 * C].rearrange(
                          "p (b c) -> p b c", b=2)[:, :, h * 64:(h + 1) * 64]
                      in03 = ppv[:, base:base + 130].rearrange(
                          "p (b c) -> p b c", b=2)[:, :, 0:64]
                      recs = rec[:, 2 * half:2 * half + 2]
                      rec3 = bass.AP(tensor=recs.tensor, offset=recs.offset,
                                     ap=[*recs.ap, [0, 64]])
                      nc.vector.tensor_mul(out3, in03, rec3)
              # tanh of head h's od columns -> compact tod [tb(8), br(2), 64]
              tod = todp.tile([128, SB * 2 * 64], BF16, tag="tod", name="tod")
              in_t = od[:].rearrange("p (a b c) -> p a b c", b=2, c=C)[
                  :, :, :, h * 64:(h + 1) * 64]
              out_t = tod[:].rearrange("p (a b c) -> p a b c", b=2, c=64)
              nc.scalar.activation(out_t, in_t, AF.Tanh)
              wg3 = wgb[:].rearrange("p (b c) -> p b c", b=2)[:, :, h * 64:(h + 1) * 64]
              for tb in range(SB):
                  t3 = tod[:, tb * 128:(tb + 1) * 128].rearrange(
                      "p (b c) -> p b c", b=2)
                  eng = nc.vector if tb % 2 == 0 else nc.gpsimd
                  eng.scalar_tensor_tensor(
                      out=t3, in0=t3, scalar=0.0, in1=wg3,
                      op0=ALU.bypass, op1=ALU.mult,
                      accum_out=zparts[tb][:, h:h + 1])
                  # incremental diff col: osa_h - dsa_h
                  o_sl = od[:, tb * 2 * C + h * 64: tb * 2 * C + (h + 1) * 64]
                  d_sl = od[:, tb * 2 * C + C + h * 64: tb * 2 * C + C + (h + 1) * 64]
                  nc.gpsimd.tensor_sub(diff[tb][:, h * 64:(h + 1) * 64], o_sl, d_sl)

          # ================= the fused schedule =================
          with tc.high_priority():
              proj_qk(0)
          for tbp in range(4):
              ppvs.append(psPV.tile([128, 260], F32, tag=f"ppv{tbp}",
                                    name=f"ppv{tbp}"))

          for h in range(H):
              prio = tc.high_priority() if h < 2 else nullcontext()
              with prio:
                  state[h] = {"sdt": [sdp.tile([128, 4 * T], BF16, tag="sdt",
                                               name=f"sdt{h}_{g}") for g in range(2)],
                              "pOs": []}
                  # front of group 0
                  for j in range(4):
                      unit_front(h, j)
                  # boundary work (prev head's skewed tail / head-0 V staging)
                  if h == 0:
                      with tc.high_priority():
                          for sb in range(4):
                              proj_v(sb)
                  else:
                      sdexp(h - 1, 1)
                      dsa_pv(h - 1, 1)
                      norms(h - 1)
                  # osa PV of group 0
                  for j in range(4):
                      pv(h, j, state[h]["pOs"][j], 0, branch=0)
                  if h == 0:
                      with tc.high_priority():
                          for sb in range(4, SB):
                              proj_v(sb)
                  # group 1: front + osa PV inline
                  for j in range(4, SB):
                      unit_front(h, j)
                      pv(h, j, state[h]["pOs"][j], 0, branch=0)
                  # this head's first dsa half
                  sdexp(h, 0)
                  dsa_pv(h, 0)
                  # stage remaining qk projections one chunk per head
                  if 1 <= h <= 5:
                      proj_qk(h, on_pool=True)
                  if h >= 1:
                      state.pop(h - 1)

          # ================= finale =================
          sdexp(H - 1, 1)
          dsa_pv(H - 1, 1)
          norms(H - 1)

          # stage C: gate + blend per t-block
          cp = ctx.enter_context(tc.tile_pool(name="stageC", bufs=2))
          zp = ctx.enter_context(tc.tile_pool(name="zP", bufs=4))
          for tb in range(SB):
              z = zp.tile([128, 1], F32, tag="z", name="z")
              nc.vector.tensor_reduce(z[:], zparts[tb][:], mybir.AxisListType.X,
                                      ALU.add)
              e = zp.tile([128, 1], F32, tag="e", name="e")
              # e = exp(-(z + bg)) = exp(-z + nbg)
              nc.scalar.activation(e[:], z[:], AF.Exp, bias=nbg_t[:], scale=-1.0)
              g = zp.tile([128, 1], F32, tag="g", name="g")
              nc.vector.tensor_scalar_add(e[:], e[:], 1.0)
              nc.vector.reciprocal(g[:], e[:])
              outt = cp.tile([128, C], F32, tag="outt", name="outt")
              eng = nc.vector if tb % 2 == 0 else nc.gpsimd
              eng.scalar_tensor_tensor(
                  out=outt[:], in0=diff[tb][:], scalar=g[:],
                  in1=od[:, tb * 2 * C + C:(tb + 1) * 2 * C],
                  op0=ALU.mult, op1=ALU.add)
              nc.sync.dma_start(out_d[tb * 128:(tb + 1) * 128, :], outt[:])

    nc.finalize()
    return nc


_CACHE = {}


def _prep_in_maps(hidden_states, dependency_matrix, Wq, bq, Wk, bk, Wv, bv, Wg, bg):
    bf = ml_dtypes.bfloat16
    hs = np.asarray(hidden_states, dtype=np.float32)
    dep = np.asarray(dependency_matrix, dtype=np.float32)
    shared = {
        "wq": np.ascontiguousarray(np.asarray(Wq, np.float32).T).astype(bf),
        "wk": np.ascontiguousarray(np.asarray(Wk, np.float32).T).astype(bf),
        "wv": np.ascontiguousarray(np.asarray(Wv, np.float32).T).astype(bf),
        "bqs": (np.asarray(bq, np.float32) * 0.125).reshape(C, 1),
        "bk": np.asarray(bk, np.float32).reshape(C, 1),
        "bv": np.ascontiguousarray(np.asarray(bv, np.float32).reshape(C)),
        "wg": np.ascontiguousarray(np.asarray(Wg, np.float32).reshape(2 * C)).astype(bf),
        "nbg": (-np.asarray(bg, np.float32)).reshape(1),
    }
    in_maps = []
    for b in range(B):
        m = dict(shared)
        m["xT"] = np.ascontiguousarray(hs[b].T).astype(bf)
        m["dep"] = np.ascontiguousarray(dep[b].T).astype(bf)
        in_maps.append(m)
    return in_maps


def kernel(**inputs):
    from concourse.bass_utils import run_bass_kernel_spmd
    if "nc" not in _CACHE:
        _CACHE["nc"] = build_nc()
    nc = _CACHE["nc"]
    in_maps = _prep_in_maps(**inputs)
    res = run_bass_kernel_spmd(nc, in_maps, core_ids=list(range(NCORES)))
    out = np.stack([res.results[i]["out"] for i in range(NCORES)], axis=0)
    return out.astype(np.float32)
